# revision 1
# baseline (speedup 1.0000x reference)
"""Single-head causal attention on 8 Trainium2 NeuronCores.

Sharding: core = 2*b + c handles batch b (of 4) and query rows {2j+c}
(1024 rows) — balanced causal work per core, no collectives (inputs are
replicated host-side).

Algebra: scores = Q@K.T = x @ (Wk.T@Wq) @ x.T and (attn@V)@Wo.T =
attn @ (x@(Wo@Wv).T), so with host-precomputed G = Wk.T@Wq and
Wvo = Wo@Wv (exact fp32) the device only runs:
  QT[i,q]   = G @ xT[:, :1024]          (1 projection)
  VWo[l,o]  = x-chunks.T @ WvoT          (1 projection)
  S.T[l,q]  = xT-chunks.T @ QT           -> expT = exp(S.T/32) * causal_mask
  out[q,o]  = (expT.T @ VWo) / (expT.T @ 1)
All matmuls bf16 with fp32 PSUM accumulate; x columns are permuted per
core so its q rows are xT cols 0..1023 and the causal structure is the
same compile-time pattern on every core (masks differ only as data).
"""

import os
import numpy as np
import ml_dtypes

import concourse.bass as bass
import concourse.bacc as bacc
import concourse.mybir as mybir
import concourse.tile as tile
from concourse.bass_utils import run_bass_kernel_spmd

BF16 = ml_dtypes.bfloat16
B, S, D = 4, 2048, 1024
NC = 8          # i-chunks of 128 (contraction dim)
NL = 16         # l-chunks of 128
# (qb, cl) score tiles that need an elementwise causal mask, per q-block
MIXED = {0: [0, 1, 2, 3, 8, 9, 10, 11], 1: [4, 5, 6, 7, 12, 13, 14, 15]}
# score chunks computed per q-block (others are fully masked -> skipped)
SCHUNKS = {0: [0, 1, 2, 3, 8, 9, 10, 11], 1: list(range(16))}

LAST_EXEC_TIME_NS = None
LAST_RESULTS = None
_CACHE = {}


def _attn_chunks(t):
    """l-chunks needed by q-tile t (128 cols): first-half 0..t, second-half 8..8+t."""
    return list(range(t + 1)) + list(range(8, 9 + t))


def _build(with_biases: bool, repeat: int = 1):
    f32, bf16 = mybir.dt.float32, mybir.dt.bfloat16
    nc = bacc.Bacc("TRN2", target_bir_lowering=False, debug=False, num_devices=8)

    xT = nc.dram_tensor("xT", [128, NC, S], bf16, kind="ExternalInput")
    wg = nc.dram_tensor("wg", [128, NC, D], bf16, kind="ExternalInput")   # G.T layout
    wvo = nc.dram_tensor("wvo", [128, NC, D], bf16, kind="ExternalInput")  # Wvo.T layout
    mk = nc.dram_tensor("mk", [2, 8, 128, 512], bf16, kind="ExternalInput")
    if with_biases:
        vqd = nc.dram_tensor("vq", [128, NC, 1], bf16, kind="ExternalInput")  # Wk.T@bq
        bvod = nc.dram_tensor("bvo", [1, D], bf16, kind="ExternalInput")      # Wo@bv+bo
    out = nc.dram_tensor("out", [1024, D], f32, kind="ExternalOutput")

    with tile.TileContext(nc) as tc:
      for _rep in range(repeat):
        with (
            tc.tile_pool(name="big", bufs=1) as big,
            tc.tile_pool(name="cst", bufs=1) as cst,
            tc.tile_pool(name="psum", bufs=1, space=bass.MemorySpace.PSUM) as psp,
        ):
            x_sb = big.tile([128, NC, S], bf16)        # x^T   [i_loc, ic, l]
            qt_sb = big.tile([128, NC, D], bf16)       # G@xTq [i_loc, ic, q]
            vwo_sb = big.tile([128, NL, D], bf16)      # x@Wvo.T [l_loc, lt, do]

            for ic in range(NC):
                nc.sync.dma_start(x_sb[:, ic, :], xT.ap()[:, ic, :])

            ones_col = cst.tile([128, 1], bf16)
            nc.vector.memset(ones_col[:], 1.0)
            if with_biases:
                ones_row = cst.tile([1, 512], bf16)
                nc.vector.memset(ones_row[:], 1.0)
                vq_sb = cst.tile([128, NC, 1], bf16)
                bvo_sb = cst.tile([1, D], bf16)
                nc.sync.dma_start(vq_sb[:], vqd.ap())
                nc.sync.dma_start(bvo_sb[:], bvod.ap())
                vxl_sb = cst.tile([1, S], bf16)

            # ---------------- phase A: the two projections ----------------
            with tc.tile_pool(name="phA", bufs=1) as pha:
                wg_sb = pha.tile([128, NC, D], bf16, tag="w", bufs=2)
                for ic in range(NC):
                    nc.sync.dma_start(wg_sb[:, ic, :], wg.ap()[:, ic, :])
                wvo_w = pha.tile([128, NC, D], bf16, tag="w", bufs=2)
                for ic in range(NC):
                    nc.sync.dma_start(wvo_w[:, ic, :], wvo.ap()[:, ic, :])

                # A1: QT = G @ xT[:, 0:1024]
                for dc in range(NC):
                    pa0 = psp.tile([128, 512], f32, tag="paw", bufs=3)
                    pa1 = psp.tile([128, 512], f32, tag="paw", bufs=3)
                    for ic in range(NC):
                        lw = wg_sb[:, ic, dc * 128:(dc + 1) * 128]
                        st, sp = ic == 0, ic == NC - 1
                        nc.tensor.matmul(pa0[:], lw, x_sb[:, ic, 0:512], start=st, stop=sp)
                        nc.tensor.matmul(pa1[:], lw, x_sb[:, ic, 512:1024], start=st, stop=sp)
                    nc.scalar.copy(qt_sb[:, dc, 0:512], pa0[:])
                    nc.scalar.copy(qt_sb[:, dc, 512:1024], pa1[:])

                # A2: VWo[l, do] = x-chunks.T @ WvoT (+ bvo)
                for lt in range(NL):
                    pa0 = psp.tile([128, 512], f32, tag="paw", bufs=3)
                    pa1 = psp.tile([128, 512], f32, tag="paw", bufs=3)
                    for ic in range(NC):
                        lw = x_sb[:, ic, lt * 128:(lt + 1) * 128]
                        st = ic == 0
                        sp = ic == NC - 1 and not with_biases
                        nc.tensor.matmul(pa0[:], lw, wvo_w[:, ic, 0:512], start=st, stop=sp)
                        nc.tensor.matmul(pa1[:], lw, wvo_w[:, ic, 512:1024], start=st, stop=sp)
                    if with_biases:
                        nc.tensor.matmul(pa0[:], ones_row[0:1, 0:128], bvo_sb[0:1, 0:512],
                                         start=False, stop=True)
                        nc.tensor.matmul(pa1[:], ones_row[0:1, 0:128], bvo_sb[0:1, 512:1024],
                                         start=False, stop=True)
                    nc.vector.tensor_copy(vwo_sb[:, lt, 0:512], pa0[:])
                    nc.vector.tensor_copy(vwo_sb[:, lt, 512:1024], pa1[:])

                # bias term bq.K[l]: vxl = (Wk.T bq).T @ xT  [1, S]
                if with_biases:
                    for lh in range(4):
                        pv = psp.tile([1, 512], f32, tag="pv", bufs=2)
                        for ic in range(NC):
                            nc.tensor.matmul(pv[:], vq_sb[:, ic, 0:1],
                                             x_sb[:, ic, lh * 512:(lh + 1) * 512],
                                             start=(ic == 0), stop=(ic == NC - 1))
                        nc.vector.tensor_copy(vxl_sb[0:1, lh * 512:(lh + 1) * 512], pv[:])

            # ---------------- phase B: attention ----------------
            with tc.tile_pool(name="phB", bufs=1) as phb:
                for qb in range(2):
                    expt = {}
                    for cl in SCHUNKS[qb]:
                        # q-cols before the diagonal are fully masked AND never
                        # read by the attn stage (its chunk lists exclude them)
                        # -> compute only the surviving q-suffix.
                        if cl in MIXED[qb]:
                            off = 128 * ((cl if cl < 8 else cl - 8) - 4 * qb)
                        else:
                            off = 0
                        qs = slice(qb * 512 + off, qb * 512 + 512)
                        ps = psp.tile([128, 512], f32, tag="paw", bufs=3, name="psS")
                        for ic in range(NC):
                            st = ic == 0
                            sp = ic == NC - 1 and not with_biases
                            nc.tensor.matmul(
                                ps[:, off:512], x_sb[:, ic, cl * 128:(cl + 1) * 128],
                                qt_sb[:, ic, qs], start=st, stop=sp)
                        if with_biases:
                            nc.tensor.matmul(ps[:, off:512],
                                             vxl_sb[0:1, cl * 128:(cl + 1) * 128],
                                             ones_row[0:1, 0:512 - off],
                                             start=False, stop=True)
                        et = phb.tile([128, 512], bf16, tag="exp", bufs=24, name="et")
                        nc.scalar.activation(et[:, off:512], ps[:, off:512],
                                             mybir.ActivationFunctionType.Exp,
                                             scale=1.0 / 32.0)
                        if cl in MIXED[qb]:
                            mi = MIXED[qb].index(cl)
                            mt = phb.tile([128, 512], bf16, tag="mask", bufs=4, name="mt")
                            nc.sync.dma_start(mt[:, off:512], mk.ap()[qb, mi, :, off:512])
                            nc.vector.tensor_tensor(et[:, off:512], et[:, off:512],
                                                    mt[:, off:512],
                                                    mybir.AluOpType.mult)
                        expt[cl] = et
                    for tl in range(4):
                        t = 4 * qb + tl
                        chunks = _attn_chunks(t)
                        po0 = psp.tile([128, 512], f32, tag="po0", bufs=2)
                        po1 = psp.tile([128, 512], f32, tag="po1", bufs=2)
                        pss = psp.tile([128, 1], f32, tag="ps", bufs=1)
                        nlast = len(chunks) - 1
                        for i, cl in enumerate(chunks):
                            lw = expt[cl][:, tl * 128:(tl + 1) * 128]
                            st, sp = i == 0, i == nlast
                            nc.tensor.matmul(po0[:], lw, vwo_sb[:, cl, 0:512], start=st, stop=sp)
                            nc.tensor.matmul(po1[:], lw, vwo_sb[:, cl, 512:1024], start=st, stop=sp)
                            nc.tensor.matmul(pss[:], lw, ones_col[:], start=st, stop=sp)
                        rec = phb.tile([128, 1], f32, tag="rec", bufs=4, name="rec")
                        nc.vector.reciprocal(rec[:], pss[:])
                        ot = phb.tile([128, 1024], f32, tag="outp", bufs=3, name="ot")
                        nc.vector.tensor_scalar_mul(ot[:, 0:512], po0[:], rec[:])
                        nc.vector.tensor_scalar_mul(ot[:, 512:1024], po1[:], rec[:])
                        nc.sync.dma_start(out.ap()[t * 128:(t + 1) * 128, 0:512],
                                          ot[:, 0:512])
                        nc.sync.dma_start(out.ap()[t * 128:(t + 1) * 128, 512:1024],
                                          ot[:, 512:1024])

    nc.compile()
    return nc


def _host_weights(Wq, Wk, Wv, Wo):
    G = (Wk.T.astype(np.float64) @ Wq.astype(np.float64)).astype(np.float32)
    Wvo = (Wo.astype(np.float64) @ Wv.astype(np.float64)).astype(np.float32)

    def wlayout(W):  # lhsT/rhs layout [i_loc, ic, d] = W[d, i] i.e. W.T chunked
        return np.ascontiguousarray(
            W.T.reshape(8, 128, D).transpose(1, 0, 2)).astype(BF16)

    # QT = G @ xT: lhsT[i, d] = G[d, i] -> wlayout(G)
    # VWo = x @ Wvo.T: rhs[i, do] = Wvo[do, i] -> wlayout(Wvo)
    return wlayout(G), wlayout(Wvo)


def _prep_inputs(x, Wq, bq, Wk, bk, Wv, bv, Wo, bo):
    wg_a, wvo_a = _host_weights(Wq, Wk, Wv, Wo)

    i = np.arange(128)[:, None]
    jl = np.arange(512)[None, :]
    masks = {}
    for c in (0, 1):
        m = np.zeros((2, 8, 128, 512), dtype=np.float32)
        for qb in (0, 1):
            j = 512 * qb + jl
            for mi, cl in enumerate(MIXED[qb]):
                if cl < 8:
                    keep = (128 * cl + i) <= j
                else:
                    keep = (128 * (cl - 8) + i) <= (j - 1 + c)
                m[qb, mi] = keep
        masks[c] = m.astype(BF16)

    with_biases = _CACHE.get("with_biases", False)
    if with_biases:
        vq = (Wk.T.astype(np.float64) @ bq.astype(np.float64)).astype(np.float32)
        vq_a = np.ascontiguousarray(vq.reshape(8, 128, 1).transpose(1, 0, 2)).astype(BF16)
        bvo = (Wo.astype(np.float64) @ bv.astype(np.float64) + bo).astype(np.float32)
        bvo_a = bvo.reshape(1, D).astype(BF16)

    in_maps = []
    for core in range(8):
        b, c = core // 2, core % 2
        perm = np.concatenate([np.arange(c, S, 2), np.arange(1 - c, S, 2)])
        xTp = x[b].T[:, perm]                                  # [D, S]
        xa = np.ascontiguousarray(
            xTp.reshape(8, 128, S).transpose(1, 0, 2)).astype(BF16)
        im = {"xT": xa, "wg": wg_a, "wvo": wvo_a, "mk": masks[c]}
        if with_biases:
            im["vq"] = vq_a
            im["bvo"] = bvo_a
        in_maps.append(im)
    return in_maps


def kernel(x, Wq, bq, Wk, bk, Wv, bv, Wo, bo):
    global LAST_EXEC_TIME_NS, LAST_RESULTS
    args = [np.asarray(a, np.float32) for a in (Wq, bq, Wk, bk, Wv, bv, Wo, bo)]
    Wq, bq, Wk, bk, Wv, bv, Wo, bo = args
    # bk shifts every score of a query row equally -> cancels in softmax.
    with_biases = any(np.any(a) for a in (bq, bv, bo))
    _CACHE["with_biases"] = with_biases
    key = ("nc", with_biases)
    if key not in _CACHE:
        _CACHE[key] = _build(with_biases)
    nc = _CACHE[key]

    x = np.asarray(x, dtype=np.float32)
    in_maps = _prep_inputs(x, Wq, bq, Wk, bk, Wv, bv, Wo, bo)

    res = run_bass_kernel_spmd(nc, in_maps, list(range(8)),
                               trace=bool(os.environ.get("BASS_TRACE")))
    LAST_EXEC_TIME_NS = res.exec_time_ns
    LAST_RESULTS = res

    full = np.empty((B, S, D), dtype=np.float32)
    for core in range(8):
        b, c = core // 2, core % 2
        full[b, c::2, :] = res.results[core]["out"]
    return full


# ---------------- numpy emulation of the device program (for testing) ----
def emulate(x, Wq, bq, Wk, bk, Wv, bv, Wo, bo, cast=True):
    def cst(a):
        return a.astype(BF16).astype(np.float32) if cast else a.astype(np.float32)

    G = (Wk.T.astype(np.float64) @ Wq.astype(np.float64)).astype(np.float32)
    Wvo = (Wo.astype(np.float64) @ Wv.astype(np.float64)).astype(np.float32)
    vq = (Wk.T.astype(np.float64) @ bq.astype(np.float64)).astype(np.float32)
    bvo = (Wo.astype(np.float64) @ bv.astype(np.float64) + bo).astype(np.float32)

    full = np.empty((B, S, D), dtype=np.float32)
    i = np.arange(128)[:, None]
    jl = np.arange(512)[None, :]
    for core in range(8):
        b, c = core // 2, core % 2
        perm = np.concatenate([np.arange(c, S, 2), np.arange(1 - c, S, 2)])
        xT = cst(x[b].T[:, perm])                        # [D, S]
        QT = cst(cst(G) @ xT[:, :1024])                  # [D, 1024]
        VWo = cst(xT.T @ cst(Wvo).T + bvo[None, :])      # [S, D]
        vxl = cst(vq) @ xT                               # [S]
        outc = np.zeros((1024, D), np.float32)
        for qb in range(2):
            j = 512 * qb + jl
            et = {}
            for cl in SCHUNKS[qb]:
                sc = xT[:, cl * 128:(cl + 1) * 128].T @ QT[:, qb * 512:(qb + 1) * 512]
                sc = sc + vxl[cl * 128:(cl + 1) * 128][:, None]
                e = cst(np.exp(sc / 32.0))
                if cl in MIXED[qb]:
                    if cl < 8:
                        keep = (128 * cl + i) <= j
                    else:
                        keep = (128 * (cl - 8) + i) <= (j - 1 + c)
                    e = e * keep
                et[cl] = e                               # [128 l, 512 q]
            for tl in range(4):
                t = 4 * qb + tl
                num = np.zeros((128, D), np.float32)
                den = np.zeros((128, 1), np.float32)
                for cl in _attn_chunks(t):
                    lw = et[cl][:, tl * 128:(tl + 1) * 128]  # [l, q128]
                    num += lw.T @ VWo[cl * 128:(cl + 1) * 128, :]
                    den += lw.T @ np.ones((128, 1), np.float32)
                outc[t * 128:(t + 1) * 128] = num / den
        full[b, c::2, :] = outc
    return full



# revision 3
# speedup vs baseline: 1.1775x; 1.1775x over previous
"""Single-head causal attention on 8 Trainium2 NeuronCores.

Sharding: core = 2*b + c handles batch b (of 4) and query rows {2j+c}
(1024 rows) — balanced causal work per core, no collectives (inputs are
replicated host-side).

Algebra: scores = Q@K.T = x @ (Wk.T@Wq) @ x.T and (attn@V)@Wo.T =
attn @ (x@(Wo@Wv).T), so with host-precomputed G = Wk.T@Wq and
Wvo = Wo@Wv (exact fp32) the device only runs:
  QT[i,q]   = G @ xT[:, :1024]          (1 projection)
  VWo[l,o]  = x-chunks.T @ WvoT          (1 projection)
  S.T[l,q]  = xT-chunks.T @ QT           -> expT = exp(S.T/32) * causal_mask
  out[q,o]  = (expT.T @ VWo) / (expT.T @ 1)
All matmuls bf16 with fp32 PSUM accumulate; x columns are permuted per
core so its q rows are xT cols 0..1023 and the causal structure is the
same compile-time pattern on every core.

v2 vs v1: causal masks generated on-device (one affine_select per score
chunk-pair + a per-core [128,128] diagonal-correction tile instead of
2 MiB of mask DMAs), batched input DMAs (8 vs 44), score chunks
computed in (cl, cl+8) pairs sharing one [128,2,512] PSUM tile and one
Exp activation, merged 1024-wide PSUM evacuations, bf16 output store
(host upcasts).
"""

import os
import numpy as np
import ml_dtypes

import concourse.bass as bass
import concourse.bacc as bacc
import concourse.mybir as mybir
import concourse.tile as tile
from concourse.bass_utils import run_bass_kernel_spmd

BF16 = ml_dtypes.bfloat16
B, S, D = 4, 2048, 1024
NC = 8          # i-chunks of 128 (contraction dim)
NL = 16         # l-chunks of 128
# score chunk-pairs (j, j+8) computed per q-block; "mixed" pairs get the
# causal mask; off = 128*(j-4*qb) = first un-masked q column in the block
PAIRS = {0: [0, 1, 2, 3], 1: [0, 1, 2, 3, 4, 5, 6, 7]}
MIXEDP = {0: [0, 1, 2, 3], 1: [4, 5, 6, 7]}

LAST_EXEC_TIME_NS = None
LAST_RESULTS = None
_CACHE = {}


def _build(with_biases: bool, repeat: int = 1):
    f32, bf16 = mybir.dt.float32, mybir.dt.bfloat16
    nc = bacc.Bacc("TRN2", target_bir_lowering=False, debug=False, num_devices=8)

    xT = nc.dram_tensor("xT", [128, NC, S], bf16, kind="ExternalInput")
    wg = nc.dram_tensor("wg", [128, NC, D], bf16, kind="ExternalInput")   # G.T layout
    wvo = nc.dram_tensor("wvo", [128, NC, D], bf16, kind="ExternalInput")  # Wvo.T layout
    dmk = nc.dram_tensor("dmk", [128, 128], bf16, kind="ExternalInput")   # diag fix
    if with_biases:
        vqd = nc.dram_tensor("vq", [128, NC, 1], bf16, kind="ExternalInput")  # Wk.T@bq
        bvod = nc.dram_tensor("bvo", [1, D], bf16, kind="ExternalInput")      # Wo@bv+bo
    out = nc.dram_tensor("out", [128, NC, 2, 512], bf16, kind="ExternalOutput")

    with tile.TileContext(nc) as tc:
      for _rep in range(repeat):
        with (
            tc.tile_pool(name="big", bufs=1) as big,
            tc.tile_pool(name="cst", bufs=1) as cst,
            tc.tile_pool(name="psum", bufs=1, space=bass.MemorySpace.PSUM) as psp,
        ):
            x_sb = big.tile([128, NC, S], bf16)          # x^T   [i_loc, ic, l]
            qt_sb = big.tile([128, NC, 2, 512], bf16)    # G@xTq [i_loc, ic, qh, q]
            vwo_sb = big.tile([128, NL, 2, 512], bf16)   # x@Wvo.T [l_loc, lt, oh, o]
            wg_sb = big.tile([128, NC, D], bf16)
            wvo_w = big.tile([128, NC, D], bf16)
            dm_sb = cst.tile([128, 128], bf16)

            # batched loads, ordered so A1 (wg @ x[:, :1024]) starts early
            nc.sync.dma_start(wg_sb[:, 0:4, :], wg.ap()[:, 0:4, :])
            nc.sync.dma_start(x_sb[:, 0:4, 0:1024], xT.ap()[:, 0:4, 0:1024])
            nc.sync.dma_start(wg_sb[:, 4:8, :], wg.ap()[:, 4:8, :])
            nc.sync.dma_start(x_sb[:, 4:8, 0:1024], xT.ap()[:, 4:8, 0:1024])
            nc.sync.dma_start(wvo_w[:], wvo.ap())
            nc.sync.dma_start(x_sb[:, 0:4, 1024:2048], xT.ap()[:, 0:4, 1024:2048])
            nc.sync.dma_start(x_sb[:, 4:8, 1024:2048], xT.ap()[:, 4:8, 1024:2048])
            nc.sync.dma_start(dm_sb[:], dmk.ap())

            ones_col = cst.tile([128, 1], bf16)
            nc.vector.memset(ones_col[:], 1.0)
            if with_biases:
                ones_row = cst.tile([1, 512], bf16)
                nc.vector.memset(ones_row[:], 1.0)
                vq_sb = cst.tile([128, NC, 1], bf16)
                bvo_sb = cst.tile([1, D], bf16)
                nc.sync.dma_start(vq_sb[:], vqd.ap())
                nc.sync.dma_start(bvo_sb[:], bvod.ap())
                vxl_sb = cst.tile([1, S], bf16)

            # ---------------- phase A: the two projections ----------------
            # A1: QT = G @ xT[:, 0:1024]
            for dc in range(NC):
                pa = psp.tile([128, 2, 512], f32, tag="ps", bufs=2, name="paA1")
                for ic in range(NC):
                    lw = wg_sb[:, ic, dc * 128:(dc + 1) * 128]
                    st, sp = ic == 0, ic == NC - 1
                    nc.tensor.matmul(pa[:, 0, :], lw, x_sb[:, ic, 0:512], start=st, stop=sp)
                    nc.tensor.matmul(pa[:, 1, :], lw, x_sb[:, ic, 512:1024], start=st, stop=sp)
                nc.scalar.copy(qt_sb[:, dc, :, :], pa[:])

            # A2: VWo[l, do] = x-chunks.T @ WvoT (+ bvo)
            for lt in range(NL):
                pa = psp.tile([128, 2, 512], f32, tag="ps", bufs=2, name="paA2")
                for ic in range(NC):
                    lw = x_sb[:, ic, lt * 128:(lt + 1) * 128]
                    st = ic == 0
                    sp = ic == NC - 1 and not with_biases
                    nc.tensor.matmul(pa[:, 0, :], lw, wvo_w[:, ic, 0:512], start=st, stop=sp)
                    nc.tensor.matmul(pa[:, 1, :], lw, wvo_w[:, ic, 512:1024], start=st, stop=sp)
                if with_biases:
                    nc.tensor.matmul(pa[:, 0, :], ones_row[0:1, 0:128], bvo_sb[0:1, 0:512],
                                     start=False, stop=True)
                    nc.tensor.matmul(pa[:, 1, :], ones_row[0:1, 0:128], bvo_sb[0:1, 512:1024],
                                     start=False, stop=True)
                nc.vector.tensor_copy(vwo_sb[:, lt, :, :], pa[:])

            # bias term bq.K[l]: vxl = (Wk.T bq).T @ xT  [1, S]
            if with_biases:
                for lh in range(4):
                    pv = psp.tile([1, 512], f32, tag="pv", bufs=2)
                    for ic in range(NC):
                        nc.tensor.matmul(pv[:], vq_sb[:, ic, 0:1],
                                         x_sb[:, ic, lh * 512:(lh + 1) * 512],
                                         start=(ic == 0), stop=(ic == NC - 1))
                    nc.vector.tensor_copy(vxl_sb[0:1, lh * 512:(lh + 1) * 512], pv[:])

            # ---------------- phase B: attention ----------------
            for qb in range(2):
                expt = {}
                for j in PAIRS[qb]:
                    # chunk pair (cl, cl+8); for mixed pairs only the q-suffix
                    # off..512 survives masking / is read by the attn stage
                    mixed = j in MIXEDP[qb]
                    off = 128 * (j - 4 * qb) if mixed else 0
                    ps = psp.tile([128, 2, 512], f32, tag="ps", bufs=2, name="psS")
                    for h in range(2):
                        cl = j + 8 * h
                        for ic in range(NC):
                            st = ic == 0
                            sp = ic == NC - 1 and not with_biases
                            nc.tensor.matmul(
                                ps[:, h, off:512], x_sb[:, ic, cl * 128:(cl + 1) * 128],
                                qt_sb[:, ic, qb, off:512], start=st, stop=sp)
                        if with_biases:
                            nc.tensor.matmul(ps[:, h, off:512],
                                             vxl_sb[0:1, cl * 128:(cl + 1) * 128],
                                             ones_row[0:1, 0:512 - off],
                                             start=False, stop=True)
                    et = big.tile([128, 2, 512], bf16, tag="exp", bufs=16, name="et")
                    nc.scalar.activation(et[:, :, off:512], ps[:, :, off:512],
                                         mybir.ActivationFunctionType.Exp,
                                         scale=1.0 / 32.0)
                    if mixed:
                        # keep iff q - i - 128*j >= 0; identical iota for both
                        # halves (half 1 uses the inclusive / c=1 rule)
                        nc.gpsimd.affine_select(
                            et[:, :, off:512], et[:, :, off:512],
                            pattern=[[0, 2], [1, 512 - off]],
                            compare_op=mybir.AluOpType.is_ge,
                            fill=0.0, base=0, channel_multiplier=-1)
                        # half 1's exact diagonal is parity-dependent: zero it
                        # (c=0) / keep it (c=1) via the per-core data tile
                        nc.vector.tensor_tensor(
                            et[:, 1, off:off + 128], et[:, 1, off:off + 128],
                            dm_sb[:], mybir.AluOpType.mult)
                    expt[j] = et
                for tl in range(4):
                    t = 4 * qb + tl
                    po = psp.tile([128, 2, 512], f32, tag="po", bufs=1, name="po")
                    pss = psp.tile([128, 1], f32, tag="pss", bufs=2, name="pss")
                    npair = t + 1
                    for i in range(npair):
                        etp = expt[i]
                        for h in range(2):
                            lw = etp[:, h, tl * 128:(tl + 1) * 128]
                            st, sp = (i == 0 and h == 0), (i == npair - 1 and h == 1)
                            nc.tensor.matmul(po[:, 0, :], lw,
                                             vwo_sb[:, i + 8 * h, 0, :], start=st, stop=sp)
                            nc.tensor.matmul(po[:, 1, :], lw,
                                             vwo_sb[:, i + 8 * h, 1, :], start=st, stop=sp)
                            nc.tensor.matmul(pss[:], lw, ones_col[:], start=st, stop=sp)
                    rec = big.tile([128, 1], f32, tag="rec", bufs=4, name="rec")
                    nc.vector.reciprocal(rec[:], pss[:])
                    ot = big.tile([128, 2, 512], bf16, tag="outp", bufs=3, name="ot")
                    nc.vector.tensor_scalar_mul(ot[:], po[:], rec[:])
                    nc.sync.dma_start(out.ap()[:, t, :, :], ot[:])

    nc.compile()
    return nc


def _host_weights(Wq, Wk, Wv, Wo):
    G = (Wk.T.astype(np.float64) @ Wq.astype(np.float64)).astype(np.float32)
    Wvo = (Wo.astype(np.float64) @ Wv.astype(np.float64)).astype(np.float32)

    def wlayout(W):  # lhsT/rhs layout [i_loc, ic, d] = W[d, i] i.e. W.T chunked
        return np.ascontiguousarray(
            W.T.reshape(8, 128, D).transpose(1, 0, 2)).astype(BF16)

    # QT = G @ xT: lhsT[i, d] = G[d, i] -> wlayout(G)
    # VWo = x @ Wvo.T: rhs[i, do] = Wvo[do, i] -> wlayout(Wvo)
    return wlayout(G), wlayout(Wvo)


def _prep_inputs(x, Wq, bq, Wk, bk, Wv, bv, Wo, bo):
    wg_a, wvo_a = _host_weights(Wq, Wk, Wv, Wo)

    eye = np.eye(128, dtype=np.float32)
    dmasks = {0: (1.0 - eye).astype(BF16), 1: np.ones((128, 128), BF16)}

    with_biases = _CACHE.get("with_biases", False)
    if with_biases:
        vq = (Wk.T.astype(np.float64) @ bq.astype(np.float64)).astype(np.float32)
        vq_a = np.ascontiguousarray(vq.reshape(8, 128, 1).transpose(1, 0, 2)).astype(BF16)
        bvo = (Wo.astype(np.float64) @ bv.astype(np.float64) + bo).astype(np.float32)
        bvo_a = bvo.reshape(1, D).astype(BF16)

    in_maps = []
    for core in range(8):
        b, c = core // 2, core % 2
        perm = np.concatenate([np.arange(c, S, 2), np.arange(1 - c, S, 2)])
        xTp = x[b].T[:, perm]                                  # [D, S]
        xa = np.ascontiguousarray(
            xTp.reshape(8, 128, S).transpose(1, 0, 2)).astype(BF16)
        im = {"xT": xa, "wg": wg_a, "wvo": wvo_a, "dmk": dmasks[c]}
        if with_biases:
            im["vq"] = vq_a
            im["bvo"] = bvo_a
        in_maps.append(im)
    return in_maps


def kernel(x, Wq, bq, Wk, bk, Wv, bv, Wo, bo):
    global LAST_EXEC_TIME_NS, LAST_RESULTS
    args = [np.asarray(a, np.float32) for a in (Wq, bq, Wk, bk, Wv, bv, Wo, bo)]
    Wq, bq, Wk, bk, Wv, bv, Wo, bo = args
    # bk shifts every score of a query row equally -> cancels in softmax.
    with_biases = any(np.any(a) for a in (bq, bv, bo))
    _CACHE["with_biases"] = with_biases
    key = ("nc", with_biases)
    if key not in _CACHE:
        _CACHE[key] = _build(with_biases)
    nc = _CACHE[key]

    x = np.asarray(x, dtype=np.float32)
    in_maps = _prep_inputs(x, Wq, bq, Wk, bk, Wv, bv, Wo, bo)

    res = run_bass_kernel_spmd(nc, in_maps, list(range(8)),
                               trace=bool(os.environ.get("BASS_TRACE")))
    LAST_EXEC_TIME_NS = res.exec_time_ns
    LAST_RESULTS = res

    full = np.empty((B, S, D), dtype=np.float32)
    for core in range(8):
        b, c = core // 2, core % 2
        oc = np.asarray(res.results[core]["out"])     # [128, 8, 2, 512] bf16
        full[b, c::2, :] = (
            oc.transpose(1, 0, 2, 3).reshape(1024, D).astype(np.float32))
    return full


# ---------------- numpy emulation of the device program (for testing) ----
def emulate(x, Wq, bq, Wk, bk, Wv, bv, Wo, bo, cast=True):
    def cst(a):
        return a.astype(BF16).astype(np.float32) if cast else a.astype(np.float32)

    G = (Wk.T.astype(np.float64) @ Wq.astype(np.float64)).astype(np.float32)
    Wvo = (Wo.astype(np.float64) @ Wv.astype(np.float64)).astype(np.float32)
    vq = (Wk.T.astype(np.float64) @ bq.astype(np.float64)).astype(np.float32)
    bvo = (Wo.astype(np.float64) @ bv.astype(np.float64) + bo).astype(np.float32)

    full = np.empty((B, S, D), dtype=np.float32)
    i = np.arange(128)[:, None]
    for core in range(8):
        b, c = core // 2, core % 2
        perm = np.concatenate([np.arange(c, S, 2), np.arange(1 - c, S, 2)])
        xT = cst(x[b].T[:, perm])                        # [D, S]
        QT = cst(cst(G) @ xT[:, :1024])                  # [D, 1024]
        VWo = cst(xT.T @ cst(Wvo).T + bvo[None, :])      # [S, D]
        vxl = cst(vq) @ xT                               # [S]
        outc = np.zeros((1024, D), np.float32)
        for qb in range(2):
            et = {}
            for j in PAIRS[qb]:
                mixed = j in MIXEDP[qb]
                off = 128 * (j - 4 * qb) if mixed else 0
                qs = np.arange(qb * 512 + off, qb * 512 + 512)
                e2 = np.zeros((128, 2, 512), np.float32)
                for h in range(2):
                    cl = j + 8 * h
                    sc = xT[:, cl * 128:(cl + 1) * 128].T @ QT[:, qs]
                    sc = sc + vxl[cl * 128:(cl + 1) * 128][:, None]
                    e = cst(np.exp(sc / 32.0))
                    if mixed:
                        keep = (qs[None, :] - i - 128 * j) >= 0
                        e = e * keep
                        if h == 1 and c == 0:
                            e[:, 0:128] = e[:, 0:128] * (
                                1.0 - np.eye(128, dtype=np.float32))
                    e2[:, h, off:512] = e
                et[j] = e2
            for tl in range(4):
                t = 4 * qb + tl
                num = np.zeros((128, D), np.float32)
                den = np.zeros((128, 1), np.float32)
                for j in range(t + 1):
                    for h in range(2):
                        lw = et[j][:, h, tl * 128:(tl + 1) * 128]
                        num += lw.T @ VWo[(j + 8 * h) * 128:(j + 8 * h + 1) * 128, :]
                        den += lw.T @ np.ones((128, 1), np.float32)
                outc[t * 128:(t + 1) * 128] = cst(num / den)
        full[b, c::2, :] = outc
    return full
